# revision 1
# baseline (speedup 1.0000x reference)
"""Trainium2 Bass kernel: batched time-domain cross-correlation.

Computes, for each of 2048 (=64x32) independent pairs (fp32):
    out[g, l] = sum_k d1[g, k + l - 301] * d2[g, k],   l in [0, 603)

Algorithm: overlap-save block correlation in a half-shift (negacyclic)
real-DFT basis, so every matmul has a *shared* stationary operand (the
transform matrices) and batches all pairs in the moving operand:

  xp = d1 zero-padded/shifted; y = d2 zero-padded.
  out[B*c + j] = sum_v corr(w_{v+c}, y_v)[j]     (j in [0, B))
    w_s = xp[B*s : B*s + 2B]  (windows, stride B, length N=2B)
    y_v = y[B*v : B*v + B]    (blocks, zero-padded to N)
  Per-block circular corr via length-N negacyclic real DFT:
    bins k: Ur[k] = sum_n u[n] cos(pi n (2k+1)/N)
            Ui[k] = -sum_n u[n] sin(pi n (2k+1)/N),  k in [0, B)
    Z = X * conj(Y):  Zr = XrYr + XiYi ; Zi = XiYr - XrYi
    z[0:B] = Minv @ [Zr; Zi]  (exact: aliasing only corrupts j > B)

Mapping: forward transforms + inverse are PE matmuls with shared
stationaries; the pointwise spectral products run on the Vector engine
with the v-sum done by segmented tensor_reduce.

Sharding: data-parallel over the 2048 pairs, 256 pairs per core, 8 cores.
"""

import math
import os
import sys

import ml_dtypes
import numpy as np

if "/opt/trn_rl_repo" not in sys.path:  # harness safety; axon site usually set
    sys.path.insert(0, "/opt/trn_rl_repo")

import concourse.bacc as bacc
import concourse.bass as bass
import concourse.mybir as mybir
import concourse.tile as tile
from concourse.bass_utils import run_bass_kernel_spmd

# ---- problem constants (hardcoded per contest contract) ----
NB_PAIRS, NCH, NT = 64, 32, 3000
LAGS = 603
SHIFT = 301  # NLAG + 1
NCORES = 8
G = (NB_PAIRS * NCH) // NCORES  # 256 pairs per core

# ---- tunables ----
B = int(os.environ.get("KB", "384"))  # lag/block granularity; N = 2B
GH = int(os.environ.get("KGH", "64"))  # pairs per g-chunk (SBUF working set)
# of every RED_FRAC product/tree ops, GP_FRAC go to GpSimd and the rest to DVE
GP_FRAC = int(os.environ.get("KGP", "1"))
RED_FRAC = int(os.environ.get("KRF", "8"))
DT_MM = mybir.dt.bfloat16  # matmul moving/stationary dtype
DT_Z = mybir.dt.bfloat16  # spectra / reduced-product dtype
DT_VE = mybir.dt.bfloat16  # elementwise product dtype (2x DVE rate)
NP_MM = ml_dtypes.bfloat16

# derived
N = 2 * B
V = math.ceil(NT / B)  # y blocks
C = math.ceil(LAGS / B)  # output lag blocks
S = V + C - 1  # x windows
SP = S  # no padding needed for bf16 matmuls
CP = C
BS = B // 128  # 128-chunks per B
NQ = N // 128  # contraction chunks of a full window
NJ = B // 128  # 128-chunks of B (bins halves / out j groups)
NR = 2 * NJ  # psum bin groups of the spectrum
U = (SP - 1) * BS + NQ  # 128-chunks in xp (covers padded windows)
NBB = U * 128
W = (V * B) // 128  # 128-chunks in y
# uneven pair-chunks sized so each chunk's x-fwd psum group fits one bank
# (gh*SP <= 512 free fp32) -> one matmul group per (r, chunk): fewest PE instrs
GHX = 512 // SP  # 56 for SP=9
_chunks = []
_g = 0
while _g < G:
    _chunks.append((_g, min(GHX, G - _g)))
    _g += min(GHX, G - _g)
GHMAX = max(gh for _, gh in _chunks)
# inverse groups aligned to chunk boundaries; small final group = short tail
_b1 = _chunks[2][0]
_b2 = _chunks[4][0]
_IGROUPS = [(0, _b1), (_b1, _b2 - _b1), (_b2, G - _b2)]

_PE_CACHE = {}
LAST_EXEC_NS = None
LAST_TRACE = None


def _matrices():
    n = np.arange(N, dtype=np.float64)[:, None]
    k = np.arange(B, dtype=np.float64)[None, :]
    theta = np.pi * n * (2 * k + 1) / N
    ffull = np.concatenate([np.cos(theta), -np.sin(theta)], axis=1)  # [N, 2B]
    minv = np.linalg.inv(ffull.T)[:B, :]  # [B, 2B]
    return ffull.astype(np.float32), minv.astype(np.float32)


def _const_tiles():
    """FW [128, NR*NQ*128]: FW[i, ((r*NQ)+q)*128 + col] = Ffull[128q+i, 128r+col]
    (r-major so each r's blocks are one contiguous DMA piece)
    MT [128, 3*NJ*NJ*128]: for zg in {Mr, Mi, -Mi}:
        MT[i, ((zg*NJ + rh)*NJ + jg)*128 + col] = M[128jg + col, 128rh + i]
    """
    ffull, minv = _matrices()
    fw = np.zeros((128, NR * NQ * 128), dtype=np.float32)
    for q in range(NQ):
        for r in range(NR):
            fw[:, (r * NQ + q) * 128 : (r * NQ + q + 1) * 128] = ffull[
                128 * q : 128 * (q + 1), 128 * r : 128 * (r + 1)
            ]
    mr = minv[:, :B]
    mi = minv[:, B:]
    mats = [mr, mi, -mi]
    mt = np.zeros((128, 3 * NJ * NJ * 128), dtype=np.float32)
    for zg in range(3):
        for rh in range(NJ):
            for jg in range(NJ):
                blk = mats[zg][128 * jg : 128 * (jg + 1), 128 * rh : 128 * (rh + 1)]
                base = ((zg * NJ + rh) * NJ + jg) * 128
                mt[:, base : base + 128] = blk.T
    return fw, mt


def build_kernel():
    nc = bacc.Bacc(
        "TRN2",
        target_bir_lowering=False,
        debug=False,
        num_devices=NCORES,
    )

    xp_d = nc.dram_tensor("xp", [128, G, U], DT_MM, kind="ExternalInput")
    yp_d = nc.dram_tensor("yp", [128, G, W], DT_MM, kind="ExternalInput")
    fw_d = nc.dram_tensor("fw", [128, NR * NQ * 128], DT_MM, kind="ExternalInput")
    mt_d = nc.dram_tensor("mt", [128, 3 * NJ * NJ * 128], DT_Z, kind="ExternalInput")
    out_d = nc.dram_tensor("out", [128, G, NJ, C], mybir.dt.float32,
                           kind="ExternalOutput")

    with tile.TileContext(nc, trace_sim=False) as tc:
        with (
            tc.tile_pool(name="const", bufs=1) as cpool,
            tc.tile_pool(name="io", bufs=2) as iopool,
            tc.tile_pool(name="spec", bufs=2) as spool,
            tc.tile_pool(name="work", bufs=3) as wpool,
            tc.tile_pool(name="zpool", bufs=1) as zpool,
            tc.tile_pool(name="psum", bufs=1, space=bass.MemorySpace.PSUM) as ppool,
        ):
            fw_t = cpool.tile([128, NR * NQ * 128], DT_MM, tag="fw")
            mt_t = cpool.tile([128, 3 * NJ * NJ * 128], DT_Z, tag="mt")
            zr = zpool.tile([128, NJ, G, CP], DT_Z, tag="zr")
            zi = zpool.tile([128, NJ, G, 2, CP], DT_Z, tag="zi")
            if CP > C:
                nc.gpsimd.memset(zr[:, :, :, C:], 0.0)
                nc.gpsimd.memset(zi[:, :, :, :, C:], 0.0)

            tt_i = 0

            def tt_eng(gpf, rdf):
                # weighted DVE/GpSimd split over all product/add ops:
                # gpf of every rdf ops go to GpSimd, rest to DVE
                nonlocal tt_i
                tt_i += 1
                if rdf and (tt_i - 1) % rdf < gpf:
                    return nc.gpsimd
                return nc.vector

            outt = iopool.tile([128, G, NJ, C], mybir.dt.float32, tag="outt", bufs=1)

            def emit_inverse(fgi):
                ig0, ign = _IGROUPS[fgi]
                gsl = slice(ig0, ig0 + ign)
                for jg in range(NJ):
                    ps = ppool.tile([128, GHMAX * 3, CP], mybir.dt.float32,
                                    tag="psC", bufs=2)
                    ps = ps[:, :ign, :]
                    nmm = 3 * NJ
                    i = 0
                    for rh in range(NJ):
                        srcs = (
                            (0, zr[:, rh, gsl, :]),
                            (1, zi[:, rh, gsl, 0, :]),
                            (2, zi[:, rh, gsl, 1, :]),
                        )
                        for zg, rhs in srcs:
                            lhsT = mt_t[
                                :,
                                ((zg * NJ + rh) * NJ + jg) * 128 :
                                ((zg * NJ + rh) * NJ + jg + 1) * 128,
                            ]
                            nc.tensor.matmul(
                                ps[:], lhsT, rhs,
                                start=(i == 0), stop=(i == nmm - 1),
                            )
                            i += 1
                    nc.scalar.copy(out=outt[:, gsl, jg, :], in_=ps[:, :, :C])
                nc.sync.dma_start(
                    out_d.ap()[:, gsl, :, :], outt[:, gsl, :, :]
                )

            inv_emitted = 0
            for ci, (g0, gh) in enumerate(_chunks):
                last = ci >= len(_chunks) - 2
                xin = iopool.tile([128, GHMAX, U], DT_MM, tag="xin", bufs=3)
                yin = iopool.tile([128, GHMAX, W], DT_MM, tag="yin", bufs=3)
                nc.sync.dma_start(xin[:, :gh, :], xp_d.ap()[:, g0 : g0 + gh, :])
                nc.sync.dma_start(yin[:, :gh, :], yp_d.ap()[:, g0 : g0 + gh, :])
                if ci == 1:
                    # mt is first needed by the first deferred inverse
                    nc.sync.dma_start(mt_t[:], mt_d.ap())
                if ci == 0:
                    # consts after the first input tiles: r-pieces in use order
                    r_order0 = [x for rh in range(NJ) for x in (rh, NJ + rh)]
                    for r in r_order0:
                        nc.sync.dma_start(
                            fw_t[:, r * NQ * 128 : (r + 1) * NQ * 128],
                            fw_d.ap()[:, r * NQ * 128 : (r + 1) * NQ * 128],
                        )

                xs = spool.tile([128, NR, GHMAX, SP], DT_VE, tag="xs")
                ys = spool.tile([128, NR, GHMAX, V], DT_VE, tag="ys")

                # ---- forward transforms, x and y interleaved per bin
                # group; r-order pairs (rh, NJ+rh) so PW group rh unblocks
                # after two r-iterations
                r_order = [x for rh in range(NJ) for x in (rh, NJ + rh)]
                for r in r_order:
                    ps = ppool.tile([128, GHMAX, SP], mybir.dt.float32, tag="psA",
                                    bufs=4)
                    for q in range(NQ):
                        lhsT = fw_t[:, (r * NQ + q) * 128 : (r * NQ + q + 1) * 128]
                        rhs = xin[:, 0:gh, q : q + BS * (SP - 1) + 1 : BS]
                        nc.tensor.matmul(
                            ps[:, :gh, :], lhsT, rhs,
                            start=(q == 0), stop=(q == NQ - 1),
                        )
                    nc.scalar.copy(out=xs[:, r, 0:gh, :], in_=ps[:, :gh, :])
                    ps = ppool.tile([128, GHMAX, V], mybir.dt.float32, tag="psB",
                                    bufs=2)
                    for q in range(NJ):
                        lhsT = fw_t[:, (r * NQ + q) * 128 : (r * NQ + q + 1) * 128]
                        rhs = yin[:, 0:gh, q : q + BS * (V - 1) + 1 : BS]
                        nc.tensor.matmul(
                            ps[:, :gh, :], lhsT, rhs,
                            start=(q == 0), stop=(q == NJ - 1),
                        )
                    nc.scalar.copy(out=ys[:, r, 0:gh, :], in_=ps[:, :gh, :])

                # deferred inverse: emit groups whose products finished in
                # prior chunks AFTER this chunk's forward matmuls, so the PE
                # queue never stalls waiting on the product engines
                while (
                    inv_emitted < len(_IGROUPS)
                    and _IGROUPS[inv_emitted][0] + _IGROUPS[inv_emitted][1] <= g0
                ):
                    emit_inverse(inv_emitted)
                    inv_emitted += 1

                # ---- pointwise products + v-sum tree (DVE + GpSimd) ----
                # bias the last chunk toward DVE (faster) to shrink the tail
                gpf, rdf = (1, 8) if last else (GP_FRAC, RED_FRAC)
                HV = V // 2
                for c in range(C):
                    for rh in range(NJ):
                        with nc.allow_low_precision("bf16 spectra products"):
                            pr = wpool.tile([128, GHMAX, 2, V], DT_VE, tag="pr",
                                            bufs=4)
                            tt_eng(gpf, rdf).tensor_mul(
                                pr[:, :gh, 0, :],
                                xs[:, rh, :gh, c : c + V],
                                ys[:, rh, :gh, :],
                            )
                            tt_eng(gpf, rdf).tensor_mul(
                                pr[:, :gh, 1, :],
                                xs[:, NJ + rh, :gh, c : c + V],
                                ys[:, NJ + rh, :gh, :],
                            )
                            # tree-sum over (2, V): stride-1 halves each pass
                            w4 = wpool.tile([128, GHMAX, 2, HV], DT_VE, tag="w4",
                                            bufs=4)
                            tt_eng(gpf, rdf).tensor_add(
                                w4[:, :gh], pr[:, :gh, :, 0:HV],
                                pr[:, :gh, :, HV:V],
                            )
                            w2 = wpool.tile([128, GHMAX, 2, HV // 2], DT_VE,
                                            tag="w2", bufs=4)
                            tt_eng(gpf, rdf).tensor_add(
                                w2[:, :gh], w4[:, :gh, :, 0 : HV // 2],
                                w4[:, :gh, :, HV // 2 : HV],
                            )
                            w1 = wpool.tile([128, GHMAX, 2], DT_VE, tag="w1",
                                            bufs=4)
                            tt_eng(gpf, rdf).tensor_add(
                                w1[:, :gh], w2[:, :gh, :, 0], w2[:, :gh, :, 1]
                            )
                            tt_eng(gpf, rdf).tensor_add(
                                zr[:, rh, g0 : g0 + gh, c], w1[:, :gh, 0],
                                w1[:, :gh, 1],
                            )
                            pr2 = wpool.tile([128, GHMAX, 2, V], DT_VE, tag="pr",
                                             bufs=4)
                            tt_eng(gpf, rdf).tensor_mul(
                                pr2[:, :gh, 0, :],
                                xs[:, NJ + rh, :gh, c : c + V],
                                ys[:, rh, :gh, :],
                            )
                            tt_eng(gpf, rdf).tensor_mul(
                                pr2[:, :gh, 1, :],
                                xs[:, rh, :gh, c : c + V],
                                ys[:, NJ + rh, :gh, :],
                            )
                            m4 = wpool.tile([128, GHMAX, 2, HV], DT_VE, tag="w4",
                                            bufs=4)
                            tt_eng(gpf, rdf).tensor_add(
                                m4[:, :gh], pr2[:, :gh, :, 0:HV],
                                pr2[:, :gh, :, HV:V],
                            )
                            m2 = wpool.tile([128, GHMAX, 2, HV // 2], DT_VE,
                                            tag="w2", bufs=4)
                            tt_eng(gpf, rdf).tensor_add(
                                m2[:, :gh], m4[:, :gh, :, 0 : HV // 2],
                                m4[:, :gh, :, HV // 2 : HV],
                            )
                            tt_eng(gpf, rdf).tensor_add(
                                zi[:, rh, g0 : g0 + gh, :, c], m2[:, :gh, :, 0],
                                m2[:, :gh, :, 1],
                            )

            while inv_emitted < len(_IGROUPS):
                emit_inverse(inv_emitted)
                inv_emitted += 1

    nc.compile()
    return nc


def _prep_core_inputs(d1f, d2f, fw, mt, core):
    """d1f/d2f: [2048, 3000] fp32. Returns the in_map for `core`."""
    sl = slice(core * G, (core + 1) * G)
    x = d1f[sl]
    y = d2f[sl]
    xp = np.zeros((G, NBB), dtype=np.float32)
    xp[:, SHIFT : SHIFT + NT] = x
    yp = np.zeros((G, V * B), dtype=np.float32)
    yp[:, :NT] = y
    # device layouts: xpT[p, g, u] = xp[g, 128u + p]
    xpT = np.ascontiguousarray(xp.reshape(G, U, 128).transpose(2, 0, 1)).astype(NP_MM)
    ypT = np.ascontiguousarray(yp.reshape(G, W, 128).transpose(2, 0, 1)).astype(NP_MM)
    return {"xp": xpT, "yp": ypT, "fw": fw.astype(NP_MM), "mt": mt.astype(NP_MM)}


def kernel(data1: np.ndarray, data2: np.ndarray) -> np.ndarray:
    import time

    d1f = np.ascontiguousarray(data1, dtype=np.float32).reshape(-1, NT)
    d2f = np.ascontiguousarray(data2, dtype=np.float32).reshape(-1, NT)
    fw, mt = _const_tiles()

    t0 = time.time()
    if "nc" not in _PE_CACHE:
        _PE_CACHE["nc"] = build_kernel()
    nc = _PE_CACHE["nc"]
    print(f"[kernel] build+compile {time.time() - t0:.1f}s", file=sys.stderr,
          flush=True)

    in_maps = [_prep_core_inputs(d1f, d2f, fw, mt, i) for i in range(NCORES)]
    t0 = time.time()
    res = run_bass_kernel_spmd(nc, in_maps, core_ids=list(range(NCORES)))
    print(f"[kernel] spmd run {time.time() - t0:.1f}s", file=sys.stderr, flush=True)
    global LAST_EXEC_NS, LAST_TRACE
    LAST_EXEC_NS = res.exec_time_ns
    LAST_TRACE = res.instructions_and_trace
    if res.exec_time_ns is not None:
        print(f"[kernel] HW exec {res.exec_time_ns} ns", file=sys.stderr, flush=True)

    outs = []
    for i in range(NCORES):
        o = res.results[i]["out"]  # [128, G, NJ, C]
        # out[g, B*c + 128*jg + p] = o[p, g, jg, c]
        full = o.transpose(1, 3, 2, 0).reshape(G, C * B)
        outs.append(full[:, :LAGS])
    return np.concatenate(outs, axis=0).reshape(NB_PAIRS, NCH, LAGS)



# revision 6
# speedup vs baseline: 1.2044x; 1.2044x over previous
"""Trainium2 Bass kernel: batched time-domain cross-correlation.

Computes, for each of 2048 (=64x32) independent pairs (fp32):
    out[g, l] = sum_k d1[g, k + l - 301] * d2[g, k],   l in [0, 603)

Algorithm: overlap-save block correlation in a half-shift (negacyclic)
real-DFT basis; every matmul has a *shared* stationary operand (the
transform matrices) and batches all pairs in the moving operand:

  xp = d1 zero-padded/shifted; y = d2 zero-padded.
  out[B*c + j] = sum_v corr(w_{v+c}, y_v)[j]     (j in [0, B))
    w_s = xp-window at stride B, length N=2B
    y_v = y[B*v : B*v + B]    (blocks, zero-padded to N)
  Per-block circular corr via length-N negacyclic real DFT:
    bins k: Ur[k] = sum_n u[n] cos(pi n (2k+1)/N)
            Ui[k] = -sum_n u[n] sin(pi n (2k+1)/N),  k in [0, B)
    Z = X * conj(Y):  Zr = XrYr + XiYi ; Zi = XiYr - XrYi
    z[0:B] = Minv @ [Zr; Zi]  (exact: aliasing only corrupts j > B)

vs the first version of this kernel:
  - xp keeps only 45 leading / 27 trailing zeros (24 chunks, not 30);
    window chunks falling in the implicit zero region are skipped
    (x-forward matmul rows 82944 -> 72192 per core).
  - Zi is a single plane (d = XiYr - XrYi subtracted on DVE), so the
    inverse has 2 stationary planes, not 3 (rows 13824 -> 7680), and
    (c=1, jg=2) lag outputs >= 640 are never computed (lags end at 602).
  - product/tree ops are fused across all 6 bin groups per op (fewer,
    fatter DVE instructions; the fixed SBUF-access cost per op is large).
  - a tunable subset of the product muls runs on GpSimd.

Sharding: data-parallel over the 2048 pairs, 256 pairs per core, 8 cores.
"""

import math
import os
import sys

import ml_dtypes
import numpy as np

if "/opt/trn_rl_repo" not in sys.path:  # harness safety; axon site usually set
    sys.path.insert(0, "/opt/trn_rl_repo")

import concourse.bacc as bacc
import concourse.bass as bass
import concourse.mybir as mybir
import concourse.tile as tile
from concourse.bass_utils import run_bass_kernel_spmd

# ---- problem constants (hardcoded per contest contract) ----
NB_PAIRS, NCH, NT = 64, 32, 3000
LAGS = 603
SHIFT = 301  # NLAG + 1
NCORES = 8
G = (NB_PAIRS * NCH) // NCORES  # 256 pairs per core

# ---- algorithm constants ----
B = 384  # lag/block granularity; N = 2B
N = 2 * B
V = 8  # y blocks (ceil 3000/384)
C = 2  # output lag blocks (ceil 603/384)
S = V + C - 1  # x windows
BS = B // 128  # 3
NQ = N // 128  # 6 contraction chunks of a full window
NJ = B // 128  # 3
NR = 2 * NJ  # 6 bin groups (Re 0..2, Im 3..5)
XLEAD = 45  # leading zeros kept in xp (301 = 2*128 + 45)
U = 24  # xp chunks: 45 + 3000 + 27 = 3072
W = 24  # y chunks: 3000 + 72 = 3072

# x-forward matmul emission: (q, s0, s1, start, stop) per (r, chunk).
# Window s uses xp chunk u = 3s + q - 2; chunks u<0 / u>=24 are implicit
# zeros (skipped).  start/stop flags per psum column range:
#   w0 first=q2 last=q5; w1-6 first=q0 last=q5; w7 first=q0 last=q4;
#   w8 first=q0 last=q1.
XMM = [
    (0, 1, 9, True, False),
    (1, 1, 9, False, False),
    (2, 0, 8, False, False),
    (3, 0, 8, False, False),
    (4, 0, 8, False, False),
    (5, 0, 7, False, True),
]

# inverse output groups: (jg, cp) — lag l = 384c + 128jg + p; lags < 603
# need (c=0, jg=0..2) and (c=1, jg=0..1).
INVJG = [(0, 2), (1, 2), (2, 1)]

DT_MM = mybir.dt.bfloat16
NP_MM = ml_dtypes.bfloat16

# pairs per chunk: psum for x-fwd is [gh, S] fp32 <= 512 -> gh <= 56
GHX = 512 // S  # 56
_chunks = []
_g = 0
while _g < G:
    _chunks.append((_g, min(GHX, G - _g)))
    _g += min(GHX, G - _g)
GHMAX = max(gh for _, gh in _chunks)
# inverse groups aligned to chunk boundaries
_b1 = _chunks[2][0]
_b2 = _chunks[4][0]
_IGROUPS = [(0, _b1), (_b1, _b2 - _b1), (_b2, G - _b2)]

# which product muls go to GpSimd: list of (c, op) with op in
# {"p2a", "p2b"}; tuned so Pool busy ~= DVE busy ~= PE busy.
_POOL_OPS = os.environ.get("KPOOL", "0:p2a,1:p2a,1:p2b")
POOL_SET = set()
if _POOL_OPS:
    for tok in _POOL_OPS.split(","):
        c_, nm = tok.split(":")
        POOL_SET.add((int(c_), nm))

_PE_CACHE = {}
LAST_EXEC_NS = None
LAST_TRACE = None


def _matrices():
    n = np.arange(N, dtype=np.float64)[:, None]
    k = np.arange(B, dtype=np.float64)[None, :]
    theta = np.pi * n * (2 * k + 1) / N
    ffull = np.concatenate([np.cos(theta), -np.sin(theta)], axis=1)  # [N, 2B]
    minv = np.linalg.inv(ffull.T)[:B, :]  # [B, 2B]
    return ffull.astype(np.float32), minv.astype(np.float32)


def _const_tiles():
    """FW [128, NR*NQ*128]: FW[i, ((r*NQ)+q)*128 + col] = Ffull[128q+i, 128r+col]
    MT [128, 2*NJ*NJ*128]: for pl in {Mr, Mi}:
        MT[i, ((pl*NJ + rh)*NJ + jg)*128 + col] = M[128jg + col, 128rh + i]
    """
    ffull, minv = _matrices()
    fw = np.zeros((128, NR * NQ * 128), dtype=np.float32)
    for q in range(NQ):
        for r in range(NR):
            fw[:, (r * NQ + q) * 128 : (r * NQ + q + 1) * 128] = ffull[
                128 * q : 128 * (q + 1), 128 * r : 128 * (r + 1)
            ]
    mr = minv[:, :B]
    mi = minv[:, B:]
    mats = [mr, mi]
    mt = np.zeros((128, 2 * NJ * NJ * 128), dtype=np.float32)
    for pl in range(2):
        for rh in range(NJ):
            for jg in range(NJ):
                blk = mats[pl][128 * jg : 128 * (jg + 1), 128 * rh : 128 * (rh + 1)]
                base = ((pl * NJ + rh) * NJ + jg) * 128
                mt[:, base : base + 128] = blk.T
    return fw, mt


def build_kernel():
    nc = bacc.Bacc(
        "TRN2",
        target_bir_lowering=False,
        debug=False,
        num_devices=NCORES,
    )

    xp_d = nc.dram_tensor("xp", [128, G, U], DT_MM, kind="ExternalInput")
    yp_d = nc.dram_tensor("yp", [128, G, W], DT_MM, kind="ExternalInput")
    fw_d = nc.dram_tensor("fw", [128, NR * NQ * 128], DT_MM, kind="ExternalInput")
    mt_d = nc.dram_tensor("mt", [128, 2 * NJ * NJ * 128], DT_MM, kind="ExternalInput")
    out_d = nc.dram_tensor("out", [128, G, NJ, C], mybir.dt.float32,
                           kind="ExternalOutput")

    with tile.TileContext(nc, trace_sim=False) as tc:
        with (
            tc.tile_pool(name="const", bufs=1) as cpool,
            tc.tile_pool(name="io", bufs=2) as iopool,
            tc.tile_pool(name="spec", bufs=2) as spool,
            tc.tile_pool(name="work", bufs=2) as wpool,
            tc.tile_pool(name="zpool", bufs=1) as zpool,
            tc.tile_pool(name="psum", bufs=1, space=bass.MemorySpace.PSUM) as ppool,
        ):
            fw_t = cpool.tile([128, NR * NQ * 128], DT_MM, tag="fw")
            mt_t = cpool.tile([128, 2 * NJ * NJ * 128], DT_MM, tag="mt")
            # spectra for all pairs: [128 bins-in-group, r-group, pair, slot]
            xs = cpool.tile([128, NR, G, S], DT_MM, tag="xs")
            ys = cpool.tile([128, NR, G, V], DT_MM, tag="ys")
            # accumulated spectral products, c-major so last free dim packs
            zr = zpool.tile([128, NJ, G, C], DT_MM, tag="zr")
            zi = zpool.tile([128, NJ, G, C], DT_MM, tag="zi")

            outt = iopool.tile([128, G, NJ, C], mybir.dt.float32, tag="outt",
                               bufs=1)

            def emit_inverse(fgi):
                ig0, ign = _IGROUPS[fgi]
                gsl = slice(ig0, ig0 + ign)
                for jg, cp in INVJG:
                    ps = ppool.tile([128, 2 * GHMAX, C], mybir.dt.float32,
                                    tag="psC", bufs=2)
                    psv = ps[:, :ign, :cp]
                    nmm = 2 * NJ
                    i = 0
                    for pl, zt in ((0, zr), (1, zi)):
                        for rh in range(NJ):
                            # moving [ign, cp]: pair-major, c inner
                            rhs = zt[:, rh, gsl, 0:cp]
                            lhsT = mt_t[
                                :,
                                ((pl * NJ + rh) * NJ + jg) * 128 :
                                ((pl * NJ + rh) * NJ + jg + 1) * 128,
                            ]
                            nc.tensor.matmul(
                                psv, lhsT, rhs,
                                start=(i == 0), stop=(i == nmm - 1),
                            )
                            i += 1
                    nc.scalar.copy(out=outt[:, gsl, jg, 0:cp], in_=psv)
                nc.sync.dma_start(
                    out_d.ap()[:, gsl, :, :], outt[:, gsl, :, :]
                )

            inv_emitted = 0
            for ci, (g0, gh) in enumerate(_chunks):
                gsl = slice(g0, g0 + gh)
                xin = iopool.tile([128, GHMAX, U], DT_MM, tag="xin", bufs=3)
                yin = iopool.tile([128, GHMAX, W], DT_MM, tag="yin", bufs=3)
                nc.sync.dma_start(xin[:, :gh, :], xp_d.ap()[:, gsl, :])
                nc.sync.dma_start(yin[:, :gh, :], yp_d.ap()[:, gsl, :])
                if ci == 1:
                    nc.sync.dma_start(mt_t[:], mt_d.ap())
                if ci == 0:
                    for r in range(NR):
                        nc.sync.dma_start(
                            fw_t[:, r * NQ * 128 : (r + 1) * NQ * 128],
                            fw_d.ap()[:, r * NQ * 128 : (r + 1) * NQ * 128],
                        )

                # ---- forward transforms, x and y interleaved per bin group
                for r in range(NR):
                    ps = ppool.tile([128, GHMAX, S], mybir.dt.float32,
                                    tag="psA", bufs=3)
                    for q, s0, s1, st, sp in XMM:
                        lhsT = fw_t[:, (r * NQ + q) * 128 : (r * NQ + q + 1) * 128]
                        u0 = 3 * s0 + q - 2
                        u1 = 3 * (s1 - 1) + q - 2
                        rhs = xin[:, 0:gh, u0 : u1 + 1 : 3]
                        nc.tensor.matmul(
                            ps[:, :gh, s0:s1], lhsT, rhs, start=st, stop=sp,
                        )
                    nc.scalar.copy(out=xs[:, r, gsl, :], in_=ps[:, :gh, :])
                    ps = ppool.tile([128, GHMAX, V], mybir.dt.float32,
                                    tag="psB", bufs=3)
                    for q in range(NJ):
                        lhsT = fw_t[:, (r * NQ + q) * 128 : (r * NQ + q + 1) * 128]
                        rhs = yin[:, 0:gh, q : q + 3 * (V - 1) + 1 : 3]
                        nc.tensor.matmul(
                            ps[:, :gh, :], lhsT, rhs,
                            start=(q == 0), stop=(q == NJ - 1),
                        )
                    nc.scalar.copy(out=ys[:, r, gsl, :], in_=ps[:, :gh, :])

                # deferred inverse AFTER this chunk's forward matmuls so the
                # PE queue never stalls on the product engines
                while (
                    inv_emitted < len(_IGROUPS)
                    and _IGROUPS[inv_emitted][0] + _IGROUPS[inv_emitted][1] <= g0
                ):
                    emit_inverse(inv_emitted)
                    inv_emitted += 1

                # ---- pointwise products + v-sum trees (DVE + GpSimd) ----
                def eng(c_, nm):
                    return nc.gpsimd if (c_, nm) in POOL_SET else nc.vector

                with nc.allow_low_precision("bf16 spectra products"):
                    for c in range(C):
                        # Zr path: sum over both r-halves and v of X*Y
                        p1 = wpool.tile([128, NR, GHMAX, V], DT_MM, tag="p1",
                                        bufs=2)
                        eng(c, "p1").tensor_mul(
                            p1[:, :, :gh, :],
                            xs[:, :, gsl, c : c + V],
                            ys[:, :, gsl, :],
                        )
                        t4 = wpool.tile([128, NR, GHMAX, V // 2], DT_MM,
                                        tag="t4", bufs=2)
                        nc.vector.tensor_add(
                            t4[:, :, :gh, :], p1[:, :, :gh, 0 : V // 2],
                            p1[:, :, :gh, V // 2 : V],
                        )
                        t2 = wpool.tile([128, NR, GHMAX, V // 4], DT_MM,
                                        tag="t2", bufs=2)
                        nc.vector.tensor_add(
                            t2[:, :, :gh, :], t4[:, :, :gh, 0 : V // 4],
                            t4[:, :, :gh, V // 4 : V // 2],
                        )
                        ta = wpool.tile([128, NJ, GHMAX, 2], DT_MM, tag="ta",
                                        bufs=2)
                        nc.vector.tensor_add(
                            ta[:, :, :gh, :], t2[:, 0:NJ, :gh, :],
                            t2[:, NJ:NR, :gh, :],
                        )
                        nc.vector.tensor_add(
                            zr[:, :, gsl, c], ta[:, :, :gh, 0], ta[:, :, :gh, 1]
                        )
                        # Zi path: d = XiYr - XrYi summed over v
                        p2a = wpool.tile([128, NJ, GHMAX, V], DT_MM, tag="p2a",
                                         bufs=2)
                        eng(c, "p2a").tensor_mul(
                            p2a[:, :, :gh, :],
                            xs[:, NJ:NR, gsl, c : c + V],
                            ys[:, 0:NJ, gsl, :],
                        )
                        p2b = wpool.tile([128, NJ, GHMAX, V], DT_MM, tag="p2b",
                                         bufs=2)
                        eng(c, "p2b").tensor_mul(
                            p2b[:, :, :gh, :],
                            xs[:, 0:NJ, gsl, c : c + V],
                            ys[:, NJ:NR, gsl, :],
                        )
                        dd = wpool.tile([128, NJ, GHMAX, V], DT_MM, tag="dd",
                                        bufs=2)
                        nc.vector.tensor_sub(
                            dd[:, :, :gh, :], p2a[:, :, :gh, :], p2b[:, :, :gh, :]
                        )
                        u4 = wpool.tile([128, NJ, GHMAX, V // 2], DT_MM,
                                        tag="u4", bufs=2)
                        nc.vector.tensor_add(
                            u4[:, :, :gh, :], dd[:, :, :gh, 0 : V // 2],
                            dd[:, :, :gh, V // 2 : V],
                        )
                        u2 = wpool.tile([128, NJ, GHMAX, V // 4], DT_MM,
                                        tag="u2", bufs=2)
                        nc.vector.tensor_add(
                            u2[:, :, :gh, :], u4[:, :, :gh, 0 : V // 4],
                            u4[:, :, :gh, V // 4 : V // 2],
                        )
                        nc.vector.tensor_add(
                            zi[:, :, gsl, c], u2[:, :, :gh, 0], u2[:, :, :gh, 1]
                        )

            while inv_emitted < len(_IGROUPS):
                emit_inverse(inv_emitted)
                inv_emitted += 1

    nc.compile()
    return nc


def _prep_core_inputs(d1f, d2f, fw, mt, core):
    """d1f/d2f: [2048, 3000] fp32. Returns the in_map for `core`."""
    sl = slice(core * G, (core + 1) * G)
    x = d1f[sl]
    y = d2f[sl]
    xp = np.zeros((G, U * 128), dtype=np.float32)
    xp[:, XLEAD : XLEAD + NT] = x
    yp = np.zeros((G, W * 128), dtype=np.float32)
    yp[:, :NT] = y
    # device layouts: xpT[p, g, u] = xp[g, 128u + p]
    xpT = np.ascontiguousarray(xp.reshape(G, U, 128).transpose(2, 0, 1)).astype(NP_MM)
    ypT = np.ascontiguousarray(yp.reshape(G, W, 128).transpose(2, 0, 1)).astype(NP_MM)
    return {"xp": xpT, "yp": ypT, "fw": fw.astype(NP_MM), "mt": mt.astype(NP_MM)}


def kernel(data1: np.ndarray, data2: np.ndarray) -> np.ndarray:
    import time

    d1f = np.ascontiguousarray(data1, dtype=np.float32).reshape(-1, NT)
    d2f = np.ascontiguousarray(data2, dtype=np.float32).reshape(-1, NT)
    fw, mt = _const_tiles()

    t0 = time.time()
    if "nc" not in _PE_CACHE:
        _PE_CACHE["nc"] = build_kernel()
    nc = _PE_CACHE["nc"]
    print(f"[kernel] build+compile {time.time() - t0:.1f}s", file=sys.stderr,
          flush=True)

    in_maps = [_prep_core_inputs(d1f, d2f, fw, mt, i) for i in range(NCORES)]
    t0 = time.time()
    res = run_bass_kernel_spmd(nc, in_maps, core_ids=list(range(NCORES)))
    print(f"[kernel] spmd run {time.time() - t0:.1f}s", file=sys.stderr, flush=True)
    global LAST_EXEC_NS, LAST_TRACE
    LAST_EXEC_NS = res.exec_time_ns
    LAST_TRACE = res.instructions_and_trace
    if res.exec_time_ns is not None:
        print(f"[kernel] HW exec {res.exec_time_ns} ns", file=sys.stderr, flush=True)

    outs = []
    for i in range(NCORES):
        o = res.results[i]["out"]  # [128, G, NJ, C]
        # out[g, 384c + 128jg + p] = o[p, g, jg, c]
        full = o.transpose(1, 3, 2, 0).reshape(G, C * NJ * 128)
        outs.append(full[:, :LAGS])
    return np.concatenate(outs, axis=0).reshape(NB_PAIRS, NCH, LAGS)


# revision 9
# speedup vs baseline: 1.2940x; 1.0744x over previous
"""Trainium2 Bass kernel: batched time-domain cross-correlation.

Computes, for each of 2048 (=64x32) independent pairs (fp32):
    out[g, l] = sum_k d1[g, k + l - 301] * d2[g, k],   l in [0, 603)

Algorithm: overlap-save block correlation in a half-shift (negacyclic)
real-DFT basis; every matmul has a *shared* stationary operand (the
transform matrices) and batches all pairs in the moving operand:

  xp = d1 zero-padded/shifted; y = d2 zero-padded.
  out[B*c + j] = sum_v corr(w_{v+c}, y_v)[j]     (j in [0, B))
    w_s = xp-window at stride B, length N=2B
    y_v = y[B*v : B*v + B]    (blocks, zero-padded to N)
  Per-block circular corr via length-N negacyclic real DFT:
    bins k: Ur[k] = sum_n u[n] cos(pi n (2k+1)/N)
            Ui[k] = -sum_n u[n] sin(pi n (2k+1)/N),  k in [0, B)
    Z = X * conj(Y):  Zr = XrYr + XiYi ; Zi = XiYr - XrYi
    z[0:B] = Minv @ [Zr; Zi]  (exact: aliasing only corrupts j > B)

vs the first version of this kernel:
  - xp keeps only 45 leading / 27 trailing zeros (24 chunks, not 30);
    window chunks falling in the implicit zero region are skipped
    (x-forward matmul rows 82944 -> 72192 per core).
  - Zi is a single plane (d = XiYr - XrYi subtracted on DVE), so the
    inverse has 2 stationary planes, not 3 (rows 13824 -> 7680), and
    (c=1, jg=2) lag outputs >= 640 are never computed (lags end at 602).
  - product/tree ops are fused across all 6 bin groups per op (fewer,
    fatter DVE instructions; the fixed SBUF-access cost per op is large).
  - a tunable subset of the product muls runs on GpSimd.

Sharding: data-parallel over the 2048 pairs, 256 pairs per core, 8 cores.
"""

import math
import os
import sys

import ml_dtypes
import numpy as np

if "/opt/trn_rl_repo" not in sys.path:  # harness safety; axon site usually set
    sys.path.insert(0, "/opt/trn_rl_repo")

import concourse.bacc as bacc
import concourse.bass as bass
import concourse.mybir as mybir
import concourse.tile as tile
from concourse.bass_utils import run_bass_kernel_spmd

# ---- problem constants (hardcoded per contest contract) ----
NB_PAIRS, NCH, NT = 64, 32, 3000
LAGS = 603
SHIFT = 301  # NLAG + 1
NCORES = 8
G = (NB_PAIRS * NCH) // NCORES  # 256 pairs per core

# ---- algorithm constants ----
B = 384  # lag/block granularity; N = 2B
N = 2 * B
V = 8  # y blocks (ceil 3000/384)
C = 2  # output lag blocks (ceil 603/384)
S = V + C - 1  # x windows
BS = B // 128  # 3
NQ = N // 128  # 6 contraction chunks of a full window
NJ = B // 128  # 3
NR = 2 * NJ  # 6 bin groups (Re 0..2, Im 3..5)
XLEAD = 45  # leading zeros kept in xp (301 = 2*128 + 45)
U = 24  # xp chunks: 45 + 3000 + 27 = 3072
W = 24  # y chunks: 3000 + 72 = 3072

# x-forward matmul emission: (q, s0, s1, start, stop) per (r, chunk).
# Window s uses xp chunk u = 3s + q - 2; chunks u<0 / u>=24 are implicit
# zeros (skipped).  start/stop flags per psum column range:
#   w0 first=q2 last=q5; w1-6 first=q0 last=q5; w7 first=q0 last=q4;
#   w8 first=q0 last=q1.
XMM = [
    (0, 1, 9, True, False),
    (1, 1, 9, False, False),
    (2, 0, 8, False, False),
    (3, 0, 8, False, False),
    (4, 0, 8, False, False),
    (5, 0, 7, False, True),
]

# inverse output groups: (jg, cp) — lag l = 384c + 128jg + p; lags < 603
# need (c=0, jg=0..2) and (c=1, jg=0..1).
INVJG = [(0, 2), (1, 2), (2, 1)]

DT_MM = mybir.dt.bfloat16
NP_MM = ml_dtypes.bfloat16

# pairs per chunk: psum for x-fwd is [gh, S] fp32 <= 512 -> gh <= 56.
# Small head chunk (overlap DMA latency + PE ramp), geometric tail so the
# product drain after the last forwards is short.
_CH = [int(t) for t in os.environ.get(
    "KCHUNKS", "32,56,56,56,28,16,8,4").split(",")]
assert sum(_CH) == G
_chunks = []
_g = 0
for _w in _CH:
    _chunks.append((_g, _w))
    _g += _w
GHMAX = max(gh for _, gh in _chunks)
# inverse groups == chunks (inverse for chunk i is emitted during chunk i+1)
_IGROUPS = list(_chunks)

# which product muls go to GpSimd: list of (c, op) with op in
# {"p1", "p2a", "p2b"}; tuned so Pool busy ~= DVE busy ~= PE busy.
# KPOOL applies to steady-state chunks, KPOOLD to the drain chunks
# (the last DRAIN_CH chunks, where products must finish fast).
def _parse_pool(s):
    out = set()
    if s:
        for tok in s.split(","):
            c_, nm = tok.split(":")
            out.add((int(c_), nm))
    return out

POOL_SET = _parse_pool(os.environ.get("KPOOL", "0:p2a,1:p2a,1:p2b"))
POOL_SET_D = _parse_pool(os.environ.get("KPOOLD", "0:p2a,1:p2b"))
DRAIN_CH = int(os.environ.get("KDRAIN", "3"))

_PE_CACHE = {}
LAST_EXEC_NS = None
LAST_TRACE = None


def _matrices():
    n = np.arange(N, dtype=np.float64)[:, None]
    k = np.arange(B, dtype=np.float64)[None, :]
    theta = np.pi * n * (2 * k + 1) / N
    ffull = np.concatenate([np.cos(theta), -np.sin(theta)], axis=1)  # [N, 2B]
    minv = np.linalg.inv(ffull.T)[:B, :]  # [B, 2B]
    return ffull.astype(np.float32), minv.astype(np.float32)


def _const_tiles():
    """FW [128, NR*NQ*128]: FW[i, ((r*NQ)+q)*128 + col] = Ffull[128q+i, 128r+col]
    MT [128, 2*NJ*NJ*128]: for pl in {Mr, Mi}:
        MT[i, ((pl*NJ + rh)*NJ + jg)*128 + col] = M[128jg + col, 128rh + i]
    """
    ffull, minv = _matrices()
    fw = np.zeros((128, NR * NQ * 128), dtype=np.float32)
    for q in range(NQ):
        for r in range(NR):
            fw[:, (r * NQ + q) * 128 : (r * NQ + q + 1) * 128] = ffull[
                128 * q : 128 * (q + 1), 128 * r : 128 * (r + 1)
            ]
    mr = minv[:, :B]
    mi = minv[:, B:]
    mats = [mr, mi]
    mt = np.zeros((128, 2 * NJ * NJ * 128), dtype=np.float32)
    for pl in range(2):
        for rh in range(NJ):
            for jg in range(NJ):
                blk = mats[pl][128 * jg : 128 * (jg + 1), 128 * rh : 128 * (rh + 1)]
                base = ((pl * NJ + rh) * NJ + jg) * 128
                mt[:, base : base + 128] = blk.T
    return fw, mt


def build_kernel():
    nc = bacc.Bacc(
        "TRN2",
        target_bir_lowering=False,
        debug=False,
        num_devices=NCORES,
    )

    xp_d = nc.dram_tensor("xp", [128, G, U], DT_MM, kind="ExternalInput")
    yp_d = nc.dram_tensor("yp", [128, G, W], DT_MM, kind="ExternalInput")
    fw_d = nc.dram_tensor("fw", [128, NR * NQ * 128], DT_MM, kind="ExternalInput")
    mt_d = nc.dram_tensor("mt", [128, 2 * NJ * NJ * 128], DT_MM, kind="ExternalInput")
    out_d = nc.dram_tensor("out", [128, G, 5], mybir.dt.float32,
                           kind="ExternalOutput")

    with tile.TileContext(nc, trace_sim=False) as tc:
        with (
            tc.tile_pool(name="const", bufs=1) as cpool,
            tc.tile_pool(name="io", bufs=2) as iopool,
            tc.tile_pool(name="spec", bufs=2) as spool,
            tc.tile_pool(name="work", bufs=2) as wpool,
            tc.tile_pool(name="zpool", bufs=1) as zpool,
            tc.tile_pool(name="psum", bufs=1, space=bass.MemorySpace.PSUM) as ppool,
        ):
            fw_t = cpool.tile([128, NR * NQ * 128], DT_MM, tag="fw")
            mt_t = cpool.tile([128, 2 * NJ * NJ * 128], DT_MM, tag="mt")
            # spectra for all pairs: [128 bins-in-group, r-group, pair, slot]
            xs = cpool.tile([128, NR, G, S], DT_MM, tag="xs")
            ys = cpool.tile([128, NR, G, V], DT_MM, tag="ys")
            # accumulated spectral products, c-major so last free dim packs
            zr = zpool.tile([128, NJ, G, C], DT_MM, tag="zr")
            zi = zpool.tile([128, NJ, G, C], DT_MM, tag="zi")

            outt = iopool.tile([128, G, 5], mybir.dt.float32, tag="outt",
                               bufs=1)

            def emit_inverse(fgi):
                ig0, ign = _IGROUPS[fgi]
                gsl = slice(ig0, ig0 + ign)
                ps = ppool.tile([128, GHMAX, 5], mybir.dt.float32,
                                tag="psC", bufs=2)
                first = True
                slot = 0
                for jg, cp in INVJG:
                    psv = ps[:, :ign, slot : slot + cp]
                    nmm = 2 * NJ
                    i = 0
                    for pl, zt in ((0, zr), (1, zi)):
                        for rh in range(NJ):
                            # moving [ign, cp]: pair-major, c inner
                            rhs = zt[:, rh, gsl, 0:cp]
                            lhsT = mt_t[
                                :,
                                ((pl * NJ + rh) * NJ + jg) * 128 :
                                ((pl * NJ + rh) * NJ + jg + 1) * 128,
                            ]
                            nc.tensor.matmul(
                                psv, lhsT, rhs,
                                start=first, stop=(i == nmm - 1),
                            )
                            first = False
                            i += 1
                    slot += cp
                nc.scalar.copy(out=outt[:, gsl, :], in_=ps[:, :ign, :])
                nc.sync.dma_start(
                    out_d.ap()[:, gsl, :], outt[:, gsl, :]
                )

            inv_emitted = 0
            for ci, (g0, gh) in enumerate(_chunks):
                gsl = slice(g0, g0 + gh)
                xin = iopool.tile([128, GHMAX, U], DT_MM, tag="xin", bufs=3)
                yin = iopool.tile([128, GHMAX, W], DT_MM, tag="yin", bufs=3)
                if ci == 0:
                    nc.sync.dma_start(
                        fw_t[:, 0 : NQ * 128], fw_d.ap()[:, 0 : NQ * 128]
                    )
                nc.sync.dma_start(xin[:, :gh, :], xp_d.ap()[:, gsl, :])
                nc.sync.dma_start(yin[:, :gh, :], yp_d.ap()[:, gsl, :])
                if ci == 0:
                    for r in range(1, NR):
                        nc.sync.dma_start(
                            fw_t[:, r * NQ * 128 : (r + 1) * NQ * 128],
                            fw_d.ap()[:, r * NQ * 128 : (r + 1) * NQ * 128],
                        )
                    nc.sync.dma_start(mt_t[:], mt_d.ap())

                # ---- forward transforms, x and y interleaved per bin
                # group; small chunks pack several r-groups per psum bank so
                # one Act copy drains several (the per-op init cost is big)
                rpb = NR
                while rpb * gh * S > 512:
                    rpb -= 1
                rpb = max(1, min(rpb, 3))
                rb = 0
                while rb < NR:
                    nx = min(rpb, NR - rb)
                    ps = ppool.tile([128, nx * gh * S], mybir.dt.float32,
                                    tag="psA", bufs=3)
                    psx = ps[:, : nx * gh * S].rearrange(
                        "p (r g s) -> p r g s", r=nx, s=S
                    )
                    for ri in range(nx):
                        r = rb + ri
                        first = ri == 0
                        for q, s0, s1, st, sp in XMM:
                            lhsT = fw_t[
                                :, (r * NQ + q) * 128 : (r * NQ + q + 1) * 128
                            ]
                            u0 = 3 * s0 + q - 2
                            u1 = 3 * (s1 - 1) + q - 2
                            rhs = xin[:, 0:gh, u0 : u1 + 1 : 3]
                            nc.tensor.matmul(
                                psx[:, ri, :, s0:s1], lhsT, rhs,
                                start=(first and st),
                                stop=(sp and ri == nx - 1),
                            )
                    nc.scalar.copy(
                        out=xs[:, rb : rb + nx, gsl, :], in_=psx[:]
                    )
                    ny = nx
                    ps = ppool.tile([128, ny * gh * V], mybir.dt.float32,
                                    tag="psB", bufs=3)
                    psy = ps[:, : ny * gh * V].rearrange(
                        "p (r g v) -> p r g v", r=ny, v=V
                    )
                    for ri in range(ny):
                        r = rb + ri
                        for q in range(NJ):
                            lhsT = fw_t[
                                :, (r * NQ + q) * 128 : (r * NQ + q + 1) * 128
                            ]
                            rhs = yin[:, 0:gh, q : q + 3 * (V - 1) + 1 : 3]
                            nc.tensor.matmul(
                                psy[:, ri, :, :], lhsT, rhs,
                                start=(ri == 0 and q == 0),
                                stop=(ri == ny - 1 and q == NJ - 1),
                            )
                    nc.scalar.copy(
                        out=ys[:, rb : rb + ny, gsl, :], in_=psy[:]
                    )
                    rb += nx

                # deferred inverse AFTER this chunk's forward matmuls so the
                # PE queue never stalls on the product engines
                while (
                    inv_emitted < len(_IGROUPS)
                    and _IGROUPS[inv_emitted][0] + _IGROUPS[inv_emitted][1] <= g0
                ):
                    emit_inverse(inv_emitted)
                    inv_emitted += 1

                # ---- pointwise products + v-sum trees (DVE + GpSimd) ----
                pset = (
                    POOL_SET_D if ci >= len(_chunks) - DRAIN_CH else POOL_SET
                )

                def eng(c_, nm):
                    return nc.gpsimd if (c_, nm) in pset else nc.vector

                with nc.allow_low_precision("bf16 spectra products"):
                    for c in range(C):
                        # Zr path: sum over both r-halves and v of X*Y
                        p1 = wpool.tile([128, NR, GHMAX, V], DT_MM, tag="p1",
                                        bufs=2)
                        eng(c, "p1").tensor_mul(
                            p1[:, :, :gh, :],
                            xs[:, :, gsl, c : c + V],
                            ys[:, :, gsl, :],
                        )
                        t4 = wpool.tile([128, NR, GHMAX, V // 2], DT_MM,
                                        tag="t4", bufs=2)
                        nc.vector.tensor_add(
                            t4[:, :, :gh, :], p1[:, :, :gh, 0 : V // 2],
                            p1[:, :, :gh, V // 2 : V],
                        )
                        t2 = wpool.tile([128, NR, GHMAX, V // 4], DT_MM,
                                        tag="t2", bufs=2)
                        nc.vector.tensor_add(
                            t2[:, :, :gh, :], t4[:, :, :gh, 0 : V // 4],
                            t4[:, :, :gh, V // 4 : V // 2],
                        )
                        ta = wpool.tile([128, NJ, GHMAX, 2], DT_MM, tag="ta",
                                        bufs=2)
                        nc.vector.tensor_add(
                            ta[:, :, :gh, :], t2[:, 0:NJ, :gh, :],
                            t2[:, NJ:NR, :gh, :],
                        )
                        nc.vector.tensor_add(
                            zr[:, :, gsl, c], ta[:, :, :gh, 0], ta[:, :, :gh, 1]
                        )
                        # Zi path: d = XiYr - XrYi summed over v
                        p2a = wpool.tile([128, NJ, GHMAX, V], DT_MM, tag="p2a",
                                         bufs=2)
                        eng(c, "p2a").tensor_mul(
                            p2a[:, :, :gh, :],
                            xs[:, NJ:NR, gsl, c : c + V],
                            ys[:, 0:NJ, gsl, :],
                        )
                        p2b = wpool.tile([128, NJ, GHMAX, V], DT_MM, tag="p2b",
                                         bufs=2)
                        eng(c, "p2b").tensor_mul(
                            p2b[:, :, :gh, :],
                            xs[:, 0:NJ, gsl, c : c + V],
                            ys[:, NJ:NR, gsl, :],
                        )
                        dd = wpool.tile([128, NJ, GHMAX, V], DT_MM, tag="dd",
                                        bufs=2)
                        nc.vector.tensor_sub(
                            dd[:, :, :gh, :], p2a[:, :, :gh, :], p2b[:, :, :gh, :]
                        )
                        u4 = wpool.tile([128, NJ, GHMAX, V // 2], DT_MM,
                                        tag="u4", bufs=2)
                        nc.vector.tensor_add(
                            u4[:, :, :gh, :], dd[:, :, :gh, 0 : V // 2],
                            dd[:, :, :gh, V // 2 : V],
                        )
                        u2 = wpool.tile([128, NJ, GHMAX, V // 4], DT_MM,
                                        tag="u2", bufs=2)
                        nc.vector.tensor_add(
                            u2[:, :, :gh, :], u4[:, :, :gh, 0 : V // 4],
                            u4[:, :, :gh, V // 4 : V // 2],
                        )
                        nc.vector.tensor_add(
                            zi[:, :, gsl, c], u2[:, :, :gh, 0], u2[:, :, :gh, 1]
                        )

            while inv_emitted < len(_IGROUPS):
                emit_inverse(inv_emitted)
                inv_emitted += 1

    nc.compile()
    return nc


def _prep_core_inputs(d1f, d2f, fw, mt, core):
    """d1f/d2f: [2048, 3000] fp32. Returns the in_map for `core`."""
    sl = slice(core * G, (core + 1) * G)
    x = d1f[sl]
    y = d2f[sl]
    xp = np.zeros((G, U * 128), dtype=np.float32)
    xp[:, XLEAD : XLEAD + NT] = x
    yp = np.zeros((G, W * 128), dtype=np.float32)
    yp[:, :NT] = y
    # device layouts: xpT[p, g, u] = xp[g, 128u + p]
    xpT = np.ascontiguousarray(xp.reshape(G, U, 128).transpose(2, 0, 1)).astype(NP_MM)
    ypT = np.ascontiguousarray(yp.reshape(G, W, 128).transpose(2, 0, 1)).astype(NP_MM)
    return {"xp": xpT, "yp": ypT, "fw": fw.astype(NP_MM), "mt": mt.astype(NP_MM)}


def kernel(data1: np.ndarray, data2: np.ndarray) -> np.ndarray:
    import time

    d1f = np.ascontiguousarray(data1, dtype=np.float32).reshape(-1, NT)
    d2f = np.ascontiguousarray(data2, dtype=np.float32).reshape(-1, NT)
    fw, mt = _const_tiles()

    t0 = time.time()
    if "nc" not in _PE_CACHE:
        _PE_CACHE["nc"] = build_kernel()
    nc = _PE_CACHE["nc"]
    print(f"[kernel] build+compile {time.time() - t0:.1f}s", file=sys.stderr,
          flush=True)

    in_maps = [_prep_core_inputs(d1f, d2f, fw, mt, i) for i in range(NCORES)]
    t0 = time.time()
    res = run_bass_kernel_spmd(nc, in_maps, core_ids=list(range(NCORES)))
    print(f"[kernel] spmd run {time.time() - t0:.1f}s", file=sys.stderr, flush=True)
    global LAST_EXEC_NS, LAST_TRACE
    LAST_EXEC_NS = res.exec_time_ns
    LAST_TRACE = res.instructions_and_trace
    if res.exec_time_ns is not None:
        print(f"[kernel] HW exec {res.exec_time_ns} ns", file=sys.stderr, flush=True)

    # slot order: (jg0,c0) (jg0,c1) (jg1,c0) (jg1,c1) (jg2,c0)
    slot_of = {(0, 0): 0, (0, 1): 1, (1, 0): 2, (1, 1): 3, (2, 0): 4}
    outs = []
    for i in range(NCORES):
        o = res.results[i]["out"]  # [128, G, 5]
        full = np.empty((G, LAGS), dtype=np.float32)
        for jg in range(NJ):
            for c in range(C):
                if (jg, c) not in slot_of:
                    continue
                lo = 384 * c + 128 * jg
                if lo >= LAGS:
                    continue
                n = min(128, LAGS - lo)
                full[:, lo : lo + n] = o[:n, :, slot_of[(jg, c)]].T
        outs.append(full)
    return np.concatenate(outs, axis=0).reshape(NB_PAIRS, NCH, LAGS)
